# revision 6
# baseline (speedup 1.0000x reference)
"""ControlNorm1D online-normalization forward, Trainium2 Bass kernel (v2, f16).

Math (per feature column l, sequential over rows t):
    scale_t = sqrt(v_t + eps);  d_t = x_t - mu_t;  out_t = d_t / scale_t
    v_{t+1}  = a*v_t + a*(1-a)*d_t^2
    mu_{t+1} = a*mu_t + (1-a)*x_t
Both mu and v are first-order linear recurrences with constant decay, so blocks
of rows become matmuls against constant triangular coefficient matrices.

v2 design (vs the f32 v1 at 239us):
  * Everything f16 except PSUM: x pretiled to f16 on host (halves HBM read),
    output DMA'd back as f16 (halves HBM write), matmul weights/moving f16.
    Accuracy cost ~5e-3 rel vs the 2e-2 gate.
  * PSUM tiles are [128, 1024] pairs (psD01 = psD0|psD1 in 2 adjacent banks),
    so each PSUM-evacuation op runs at FD=1024, amortizing the ~200-cycle
    fixed overhead. 2 bufs each for psD01/psV01 = all 8 PSUM banks.
  * Elementwise work split by engine roofline (ACT 1.2G, DVE 0.96G with 2x
    f16 mode, Pool 1.2G*0.42):
      ACT : cast psD01->dd01 (f16) + Rsqrt(psV01+eps)->r01   (~2.1us/pair)
      DVE : carry copies (mu, v) + squares dd*dd (f16 2x)    (~1.9us/pair)
      Pool: out-muls dd*r (f16) (7 of 8; every 8th on DVE)   (~1.9us/pair)
  * Software pipeline: pair i's D-phase (D-matmuls, cast, square) runs 2
    steps ahead of its V-phase (V-matmuls, rsqrt, out-mul), giving the
    cast->square chain slack so the PE never waits on it.
  * Big DMAs triggered from the (otherwise idle) Sync engine.

The feature dim L=4096 is sharded across 8 cores (512 each, no cross-core
communication). Host-side, each core's x shard is PRE-TILED to the exact SBUF
layout [128 partitions, 65 blocks * 512] so device DMAs have one big
contiguous run per partition. Outputs are written back over the same SBUF
buffer and un-pretiled on host.
"""

import numpy as np

AFWD = 0.999
EPS = 1e-5
N_ROWS = 8192
L_FULL = 4096
N_CORES = 8
LC = L_FULL // N_CORES  # 512 features per core
B = 127                 # rows per block (partition 1+t holds row t)
NBLK = 65               # 64 full blocks + 1 short (64 rows)
NPAIR = 32              # paired blocks; block 64 handled as a single tail

_f32 = np.float32
_f16 = np.float16

# DMA chunking (blocks): small first chunk primes the pipeline fast
CHUNKS_IN = [(0, 4), (4, 17), (17, 30), (30, 43), (43, 56), (56, 65)]
CHUNKS_OUT = [(0, 13), (13, 26), (26, 39), (39, 52), (52, 61), (61, 65)]


def _tri(me, ve):
    a = AFWD
    L = np.zeros((128, 128))
    for t in range(127):
        for s in range(127):
            if s == t:
                L[1 + s, 1 + t] += me
            if s < t:
                L[1 + s, 1 + t] += ve * a ** (t - 1 - s)
    return L


def _build_mats():
    a = AFWD
    LD_ev = _tri(1.0, -(1 - a))
    LV_ev = _tri(0.0, a * (1 - a))
    for t in range(127):
        LD_ev[0, 1 + t] = a**t
        LV_ev[0, 1 + t] = a**t
    LD_x = np.zeros((128, 128))
    LV_x = np.zeros((128, 128))
    for t in range(127):
        LD_x[0, 1 + t] = a ** (127 + t)
        LV_x[0, 1 + t] = a ** (127 + t)
        for s in range(127):
            LD_x[1 + s, 1 + t] = -(1 - a) * a ** (127 + t - 1 - s)
            LV_x[1 + s, 1 + t] = a * (1 - a) * a ** (127 + t - 1 - s)
    LD_x[0, 0] = a**254
    LV_x[0, 0] = a**254
    for s in range(127):
        LD_x[1 + s, 0] = -(1 - a) * a ** (253 - s)
        LV_x[1 + s, 0] = a * (1 - a) * a ** (253 - s)
    LD_od = _tri(1.0, -(1 - a))
    LV_od = _tri(0.0, a * (1 - a))
    for s in range(127):
        LD_od[1 + s, 0] = -(1 - a) * a ** (126 - s)
        LV_od[1 + s, 0] = a * (1 - a) * a ** (126 - s)
    # LD_ev/LV_ev double as the tail-block matrices (identical construction)
    mats = [LD_ev, LD_x, LD_od, LV_ev, LV_x, LV_od]
    return np.stack([m.astype(_f32) for m in mats]).astype(_f16)  # [6,128,128]


def _pretile(x_c, m_c):
    """f16 [8192, LC] -> [128, NBLK*LC]: partition 1+t of block-slice i = row
    i*127+t. Partition 0 of block 0 carries the virtual row -m."""
    xp = np.zeros((128, NBLK * LC), _f16)
    full = x_c[: 64 * B].reshape(64, B, LC).transpose(1, 0, 2)  # [127, 64, LC]
    xp[1:128, : 64 * LC] = full.reshape(B, 64 * LC)
    xp[1:65, 64 * LC :] = x_c[64 * B :]
    xp[0, :LC] = m_c  # pre-negated by caller
    return xp


def _unpretile(op):
    """Inverse of _pretile for the (f16) output buffer -> f32 [8192, LC]."""
    op = np.asarray(op).astype(_f32)
    out = np.empty((N_ROWS, LC), _f32)
    out[: 64 * B] = (
        op[1:128, : 64 * LC].reshape(B, 64, LC).transpose(1, 0, 2).reshape(-1, LC)
    )
    out[64 * B :] = op[1:65, 64 * LC :]
    return out


_PROGRAM_CACHE: dict = {}


def _raw_act(eng, out, in_, func, bias_ap, scale, mybir):
    ins = [
        eng.lower_ap(in_),
        eng.lower_ap(bias_ap),
        mybir.ImmediateValue(dtype=mybir.dt.float32, value=float(scale)),
        mybir.ImmediateValue(dtype=mybir.dt.float32, value=0.0),
    ]
    return eng.add_instruction(
        mybir.InstActivation(
            name=eng.bass.get_next_instruction_name(),
            func=func,
            ins=ins,
            outs=[eng.lower_ap(out)],
        )
    )


def _build_program():
    if "nc" in _PROGRAM_CACHE:
        return _PROGRAM_CACHE["nc"]

    import concourse.bacc as bacc
    import concourse.tile as tile
    from concourse import mybir

    nc = bacc.Bacc(
        "TRN2",
        target_bir_lowering=False,
        debug=False,
        enable_asserts=False,
        num_devices=N_CORES,
    )
    f32 = mybir.dt.float32
    f16 = mybir.dt.float16

    xp_d = nc.dram_tensor("xp", [128, NBLK * LC], f16, kind="ExternalInput").ap()
    var_d = nc.dram_tensor("var", [LC], f16, kind="ExternalInput").ap()
    mats_d = nc.dram_tensor("mats", [6, 128, 128], f16, kind="ExternalInput").ap()
    op_d = nc.dram_tensor("op", [128, NBLK * LC], f16, kind="ExternalOutput").ap()

    with tile.TileContext(nc) as tc:
        with (
            tc.tile_pool(name="consts", bufs=1) as consts,
            tc.tile_pool(name="work", bufs=1) as work,
            tc.tile_pool(name="dd", bufs=4) as dd_pool,
            tc.tile_pool(name="d2", bufs=4) as d2_pool,
            tc.tile_pool(name="rs", bufs=3) as r_pool,
            tc.tile_pool(name="psD", bufs=2, space="PSUM") as psD_pool,
            tc.tile_pool(name="psV", bufs=2, space="PSUM") as psV_pool,
        ):
            mat_tiles = []
            for mi in range(6):
                mt = consts.tile([128, 128], f16, tag=f"mat{mi}")
                nc.gpsimd.dma_start(out=mt[:], in_=mats_d[mi, :, :])
                mat_tiles.append(mt)
            eps_t = consts.tile([128, 1], f32)
            nc.vector.memset(eps_t[:], EPS)
            v0_t = consts.tile([1, LC], f16)
            nc.gpsimd.dma_start(out=v0_t[:], in_=var_d[None, :])

            W = work.tile([128, NBLK * LC], f16)
            for (b0, b1) in CHUNKS_IN:
                nc.sync.dma_start(
                    out=W[:, b0 * LC : b1 * LC], in_=xp_d[:, b0 * LC : b1 * LC]
                )

            LD_ev, LD_x, LD_od, LV_ev, LV_x, LV_od = (mt[:] for mt in mat_tiles)

            # PE warm-up: the HAM clock gate only lifts to 2.4 GHz after
            # ~3.4us of *sustained* PE activity, and the per-pair MM bursts
            # are shorter than that — without this the whole kernel runs at
            # the cold 1.2 GHz rate. Burn ~5us of back-to-back dummy matmuls
            # into a scratch PSUM tile while the first x chunk DMA is still
            # in flight; after the flip, inter-burst gaps stay below the
            # ~3.4us idle window so the PE never re-throttles.
            warm_rhs = consts.tile([128, LC], f16)
            nc.vector.memset(warm_rhs[:], 0.0)
            warm_ps = psD_pool.tile([128, 2 * LC], f32, tag="psD01")
            for _ in range(12):
                nc.tensor.matmul(
                    warm_ps[:, :LC], LD_ev, warm_rhs[:], start=True, stop=True
                )

            out_chunk_done = [False] * len(CHUNKS_OUT)

            def emit_out_dma(upto_block):
                for ci, (b0, b1) in enumerate(CHUNKS_OUT):
                    if not out_chunk_done[ci] and b1 <= upto_block:
                        nc.sync.dma_start(
                            out=op_d[:, b0 * LC : b1 * LC],
                            in_=W[:, b0 * LC : b1 * LC],
                        )
                        out_chunk_done[ci] = True

            # per-pair state carried between the D-phase and V-phase
            psD = [None] * (NPAIR + 1)
            psV = [None] * (NPAIR + 1)
            dd = [None] * (NPAIR + 1)
            d2 = [None] * (NPAIR + 1)

            def d_phase(i):
                s0 = slice(2 * i * LC, (2 * i + 1) * LC)
                s1 = slice((2 * i + 1) * LC, (2 * i + 2) * LC)
                if i > 0:  # mu carry: psD1(i-1) row0 -> W row0 of this R0
                    nc.vector.tensor_copy(
                        out=W[0:1, s0], in_=psD[i - 1][0:1, LC : 2 * LC]
                    )
                p = psD_pool.tile([128, 2 * LC], f32, tag="psD01")
                psD[i] = p
                nc.tensor.matmul(p[:, :LC], LD_ev, W[:, s0], start=True, stop=True)
                nc.tensor.matmul(p[:, LC:], LD_x, W[:, s0], start=True, stop=False)
                nc.tensor.matmul(p[:, LC:], LD_od, W[:, s1], start=False, stop=True)
                t_dd = dd_pool.tile([128, 2 * LC], f16, tag="dd01")
                dd[i] = t_dd
                nc.scalar.copy(out=t_dd[:, :], in_=p[:, :])  # f32 PSUM -> f16
                t_d2 = d2_pool.tile([128, 2 * LC], f16, tag="d201")
                d2[i] = t_d2
                nc.vector.tensor_mul(out=t_d2[:, :], in0=t_dd[:, :], in1=t_dd[:, :])

            def v_phase(j):
                s0 = slice(2 * j * LC, (2 * j + 1) * LC)
                s1 = slice((2 * j + 1) * LC, (2 * j + 2) * LC)
                t_d2 = d2[j]
                if j == 0:  # v carry seed
                    nc.vector.tensor_copy(out=t_d2[0:1, 0:LC], in_=v0_t[:])
                else:
                    nc.vector.tensor_copy(
                        out=t_d2[0:1, 0:LC], in_=psV[j - 1][0:1, LC : 2 * LC]
                    )
                p = psV_pool.tile([128, 2 * LC], f32, tag="psV01")
                psV[j] = p
                nc.tensor.matmul(p[:, :LC], LV_ev, t_d2[:, :LC], start=True, stop=True)
                nc.tensor.matmul(p[:, LC:], LV_x, t_d2[:, :LC], start=True, stop=False)
                nc.tensor.matmul(p[:, LC:], LV_od, t_d2[:, LC:], start=False, stop=True)
                r01 = r_pool.tile([128, 2 * LC], f16, tag="r01")
                _raw_act(
                    nc.scalar, r01[:, :], p[:, :],
                    mybir.ActivationFunctionType.Rsqrt, eps_t[:, :], 1.0, mybir,
                )
                mul_eng = nc.vector if (j % 8 == 7) else nc.gpsimd
                mul_eng.tensor_mul(
                    out=W[:, 2 * j * LC : (2 * j + 2) * LC],
                    in0=dd[j][:, :], in1=r01[:, :],
                )
                emit_out_dma(2 * j + 2)

            for step in range(NPAIR + 2):
                jj = step - 2
                if 0 <= jj < NPAIR:
                    v_phase(jj)
                if step < NPAIR:
                    d_phase(step)

            # tail: block 64 (64 rows; pretile zero-padded the rest)
            st = slice(64 * LC, 65 * LC)
            nc.vector.tensor_copy(out=W[0:1, st], in_=psD[NPAIR - 1][0:1, LC : 2 * LC])
            pDt = psD_pool.tile([128, 2 * LC], f32, tag="psD01")
            nc.tensor.matmul(pDt[:, :LC], LD_ev, W[:, st], start=True, stop=True)
            ddt = dd_pool.tile([128, 2 * LC], f16, tag="dd01")
            nc.scalar.copy(out=ddt[:, :LC], in_=pDt[:, :LC])
            d2t = d2_pool.tile([128, 2 * LC], f16, tag="d201")
            nc.vector.tensor_mul(out=d2t[:, :LC], in0=ddt[:, :LC], in1=ddt[:, :LC])
            nc.vector.tensor_copy(
                out=d2t[0:1, 0:LC], in_=psV[NPAIR - 1][0:1, LC : 2 * LC]
            )
            pVt = psV_pool.tile([128, 2 * LC], f32, tag="psV01")
            nc.tensor.matmul(pVt[:, :LC], LV_ev, d2t[:, :LC], start=True, stop=True)
            rt = r_pool.tile([128, 2 * LC], f16, tag="r01")
            _raw_act(
                nc.scalar, rt[:, :LC], pVt[:, :LC],
                mybir.ActivationFunctionType.Rsqrt, eps_t[:, :], 1.0, mybir,
            )
            nc.vector.tensor_mul(out=W[:, st], in0=ddt[:, :LC], in1=rt[:, :LC])
            emit_out_dma(NBLK)

    nc.compile()
    _PROGRAM_CACHE["nc"] = nc
    return nc


def kernel(x: np.ndarray, m: np.ndarray, var: np.ndarray) -> np.ndarray:
    from concourse.bass_utils import run_bass_kernel_spmd

    x = np.asarray(x, dtype=_f32)
    m = np.ascontiguousarray(np.asarray(m, dtype=_f32))
    var = np.ascontiguousarray(np.asarray(var, dtype=_f32))
    assert x.shape == (N_ROWS, L_FULL), x.shape

    nc = _build_program()
    mats = _build_mats()

    x_bf = x.astype(_f16)
    negm_bf = (-m).astype(_f16)
    var_bf = var.astype(_f16)

    in_maps = []
    for c in range(N_CORES):
        sl = slice(c * LC, (c + 1) * LC)
        in_maps.append(
            {
                "xp": _pretile(np.ascontiguousarray(x_bf[:, sl]), negm_bf[sl]),
                "var": np.ascontiguousarray(var_bf[sl]),
                "mats": mats,
            }
        )

    res = run_bass_kernel_spmd(nc, in_maps, core_ids=list(range(N_CORES)))
    out = np.concatenate(
        [_unpretile(res.results[c]["op"]) for c in range(N_CORES)], axis=1
    )
    return out.astype(_f32, copy=False)


# revision 7
# speedup vs baseline: 1.2502x; 1.2502x over previous
"""ControlNorm1D online-normalization forward, Trainium2 Bass kernel (v2, f16).

Math (per feature column l, sequential over rows t):
    scale_t = sqrt(v_t + eps);  d_t = x_t - mu_t;  out_t = d_t / scale_t
    v_{t+1}  = a*v_t + a*(1-a)*d_t^2
    mu_{t+1} = a*mu_t + (1-a)*x_t
Both mu and v are first-order linear recurrences with constant decay, so blocks
of rows become matmuls against constant triangular coefficient matrices.

v2 design (vs the f32 v1 at 239us):
  * Everything f16 except PSUM: x pretiled to f16 on host (halves HBM read),
    output DMA'd back as f16 (halves HBM write), matmul weights/moving f16.
    Accuracy cost ~5e-3 rel vs the 2e-2 gate.
  * PSUM tiles are [128, 1024] pairs (psD01 = psD0|psD1 in 2 adjacent banks),
    so each PSUM-evacuation op runs at FD=1024, amortizing the ~200-cycle
    fixed overhead. 2 bufs each for psD01/psV01 = all 8 PSUM banks.
  * Elementwise work split by engine roofline (ACT 1.2G, DVE 0.96G with 2x
    f16 mode, Pool 1.2G*0.42):
      ACT : cast psD01->dd01 (f16) + Rsqrt(psV01+eps)->r01   (~2.1us/pair)
      DVE : carry copies (mu, v) + squares dd*dd (f16 2x)    (~1.9us/pair)
      Pool: out-muls dd*r (f16) (7 of 8; every 8th on DVE)   (~1.9us/pair)
  * Software pipeline: pair i's D-phase (D-matmuls, cast, square) runs 2
    steps ahead of its V-phase (V-matmuls, rsqrt, out-mul), giving the
    cast->square chain slack so the PE never waits on it.
  * Big DMAs triggered from the (otherwise idle) Sync engine.

The feature dim L=4096 is sharded across 8 cores (512 each, no cross-core
communication). Host-side, each core's x shard is PRE-TILED to the exact SBUF
layout [128 partitions, 65 blocks * 512] so device DMAs have one big
contiguous run per partition. Outputs are written back over the same SBUF
buffer and un-pretiled on host.
"""

import numpy as np

AFWD = 0.999
EPS = 1e-5
N_ROWS = 8192
L_FULL = 4096
N_CORES = 8
LC = L_FULL // N_CORES  # 512 features per core
B = 127                 # rows per block (partition 1+t holds row t)
NBLK = 65               # 64 full blocks + 1 short (64 rows)
NPAIR = 32              # paired blocks; block 64 handled as a single tail

_f32 = np.float32
_f16 = np.float16

# DMA chunking (blocks): small first chunk primes the pipeline fast
CHUNKS_IN = [(0, 4), (4, 17), (17, 30), (30, 43), (43, 56), (56, 65)]
CHUNKS_OUT = [(0, 13), (13, 26), (26, 39), (39, 52), (52, 61), (61, 65)]


def _tri(me, ve):
    a = AFWD
    L = np.zeros((128, 128))
    for t in range(127):
        for s in range(127):
            if s == t:
                L[1 + s, 1 + t] += me
            if s < t:
                L[1 + s, 1 + t] += ve * a ** (t - 1 - s)
    return L


def _build_mats():
    a = AFWD
    LD_ev = _tri(1.0, -(1 - a))
    LV_ev = _tri(0.0, a * (1 - a))
    for t in range(127):
        LD_ev[0, 1 + t] = a**t
        LV_ev[0, 1 + t] = a**t
    LD_x = np.zeros((128, 128))
    LV_x = np.zeros((128, 128))
    for t in range(127):
        LD_x[0, 1 + t] = a ** (127 + t)
        LV_x[0, 1 + t] = a ** (127 + t)
        for s in range(127):
            LD_x[1 + s, 1 + t] = -(1 - a) * a ** (127 + t - 1 - s)
            LV_x[1 + s, 1 + t] = a * (1 - a) * a ** (127 + t - 1 - s)
    LD_x[0, 0] = a**254
    LV_x[0, 0] = a**254
    for s in range(127):
        LD_x[1 + s, 0] = -(1 - a) * a ** (253 - s)
        LV_x[1 + s, 0] = a * (1 - a) * a ** (253 - s)
    LD_od = _tri(1.0, -(1 - a))
    LV_od = _tri(0.0, a * (1 - a))
    for s in range(127):
        LD_od[1 + s, 0] = -(1 - a) * a ** (126 - s)
        LV_od[1 + s, 0] = a * (1 - a) * a ** (126 - s)
    # LD_ev/LV_ev double as the tail-block matrices (identical construction)
    mats = [LD_ev, LD_x, LD_od, LV_ev, LV_x, LV_od]
    return np.stack([m.astype(_f32) for m in mats]).astype(_f16)  # [6,128,128]


def _pretile(x_c, m_c):
    """f16 [8192, LC] -> [128, NBLK*LC]: partition 1+t of block-slice i = row
    i*127+t. Partition 0 of block 0 carries the virtual row -m."""
    xp = np.zeros((128, NBLK * LC), _f16)
    full = x_c[: 64 * B].reshape(64, B, LC).transpose(1, 0, 2)  # [127, 64, LC]
    xp[1:128, : 64 * LC] = full.reshape(B, 64 * LC)
    xp[1:65, 64 * LC :] = x_c[64 * B :]
    xp[0, :LC] = m_c  # pre-negated by caller
    return xp


def _unpretile(op):
    """Inverse of _pretile for the (f16) output buffer -> f32 [8192, LC]."""
    op = np.asarray(op).astype(_f32)
    out = np.empty((N_ROWS, LC), _f32)
    out[: 64 * B] = (
        op[1:128, : 64 * LC].reshape(B, 64, LC).transpose(1, 0, 2).reshape(-1, LC)
    )
    out[64 * B :] = op[1:65, 64 * LC :]
    return out


_PROGRAM_CACHE: dict = {}


def _raw_act(eng, out, in_, func, bias_ap, scale, mybir):
    ins = [
        eng.lower_ap(in_),
        eng.lower_ap(bias_ap),
        mybir.ImmediateValue(dtype=mybir.dt.float32, value=float(scale)),
        mybir.ImmediateValue(dtype=mybir.dt.float32, value=0.0),
    ]
    return eng.add_instruction(
        mybir.InstActivation(
            name=eng.bass.get_next_instruction_name(),
            func=func,
            ins=ins,
            outs=[eng.lower_ap(out)],
        )
    )


def _build_program():
    if "nc" in _PROGRAM_CACHE:
        return _PROGRAM_CACHE["nc"]

    import concourse.bacc as bacc
    import concourse.tile as tile
    from concourse import mybir

    nc = bacc.Bacc(
        "TRN2",
        target_bir_lowering=False,
        debug=False,
        enable_asserts=False,
        num_devices=N_CORES,
    )
    f32 = mybir.dt.float32
    f16 = mybir.dt.float16

    xp_d = nc.dram_tensor("xp", [128, NBLK * LC], f16, kind="ExternalInput").ap()
    var_d = nc.dram_tensor("var", [LC], f16, kind="ExternalInput").ap()
    mats_d = nc.dram_tensor("mats", [6, 128, 128], f16, kind="ExternalInput").ap()
    op_d = nc.dram_tensor("op", [128, NBLK * LC], f16, kind="ExternalOutput").ap()

    with tile.TileContext(nc) as tc:
        with (
            tc.tile_pool(name="consts", bufs=1) as consts,
            tc.tile_pool(name="work", bufs=1) as work,
            tc.tile_pool(name="dd", bufs=4) as dd_pool,
            tc.tile_pool(name="d2", bufs=4) as d2_pool,
            tc.tile_pool(name="rs", bufs=3) as r_pool,
            tc.tile_pool(name="psD", bufs=2, space="PSUM") as psD_pool,
            tc.tile_pool(name="psV", bufs=2, space="PSUM") as psV_pool,
        ):
            mat_tiles = []
            for mi in range(6):
                mt = consts.tile([128, 128], f16, tag=f"mat{mi}")
                nc.gpsimd.dma_start(out=mt[:], in_=mats_d[mi, :, :])
                mat_tiles.append(mt)
            eps_t = consts.tile([128, 1], f32)
            nc.vector.memset(eps_t[:], EPS)
            v0_t = consts.tile([1, LC], f16)
            nc.gpsimd.dma_start(out=v0_t[:], in_=var_d[None, :])

            W = work.tile([128, NBLK * LC], f16)
            for (b0, b1) in CHUNKS_IN:
                nc.sync.dma_start(
                    out=W[:, b0 * LC : b1 * LC], in_=xp_d[:, b0 * LC : b1 * LC]
                )

            LD_ev, LD_x, LD_od, LV_ev, LV_x, LV_od = (mt[:] for mt in mat_tiles)

            # PE warm-up: the HAM clock gate only lifts to 2.4 GHz after
            # ~3.4us of *sustained* PE activity, and the per-pair MM bursts
            # are shorter than that — without this the whole kernel runs at
            # the cold 1.2 GHz rate. Burn ~5us of back-to-back dummy matmuls
            # into a scratch PSUM tile while the first x chunk DMA is still
            # in flight; after the flip, inter-burst gaps stay below the
            # ~3.4us idle window so the PE never re-throttles.
            warm_rhs = consts.tile([128, LC], f16)
            nc.vector.memset(warm_rhs[:], 0.0)
            warm_ps = psD_pool.tile([128, 2 * LC], f32, tag="psD01")
            for _ in range(12):
                nc.tensor.matmul(
                    warm_ps[:, :LC], LD_ev, warm_rhs[:], start=True, stop=True
                )

            out_chunk_done = [False] * len(CHUNKS_OUT)

            def emit_out_dma(upto_block):
                for ci, (b0, b1) in enumerate(CHUNKS_OUT):
                    if not out_chunk_done[ci] and b1 <= upto_block:
                        nc.sync.dma_start(
                            out=op_d[:, b0 * LC : b1 * LC],
                            in_=W[:, b0 * LC : b1 * LC],
                        )
                        out_chunk_done[ci] = True

            # per-pair state carried between the D-phase and V-phase
            psD = [None] * (NPAIR + 1)
            psV = [None] * (NPAIR + 1)
            dd = [None] * (NPAIR + 1)
            d2 = [None] * (NPAIR + 1)

            def d_phase(i):
                # The mu-carry copy (cp_mu) and the next pair's D-matmuls form
                # the critical dependency ring; cp_mu for pair i+1 is emitted
                # here, AHEAD of this pair's square in the DVE queue, so the
                # ring never routes through ACT's cast.
                s0 = slice(2 * i * LC, (2 * i + 1) * LC)
                s1 = slice((2 * i + 1) * LC, (2 * i + 2) * LC)
                p = psD_pool.tile([128, 2 * LC], f32, tag="psD01")
                psD[i] = p
                nc.tensor.matmul(p[:, :LC], LD_ev, W[:, s0], start=True, stop=True)
                nc.tensor.matmul(p[:, LC:], LD_x, W[:, s0], start=True, stop=False)
                nc.tensor.matmul(p[:, LC:], LD_od, W[:, s1], start=False, stop=True)
                if i + 1 < NPAIR:  # mu carry -> W row0 of next pair's R0
                    s0n = slice(2 * (i + 1) * LC, (2 * (i + 1) + 1) * LC)
                    nc.vector.tensor_copy(out=W[0:1, s0n], in_=p[0:1, LC : 2 * LC])
                t_dd = dd_pool.tile([128, 2 * LC], f16, tag="dd01")
                dd[i] = t_dd
                nc.scalar.copy(out=t_dd[:, :], in_=p[:, :])  # f32 PSUM -> f16
                t_d2 = d2_pool.tile([128, 2 * LC], f16, tag="d201")
                d2[i] = t_d2
                nc.vector.tensor_mul(out=t_d2[:, :], in0=t_dd[:, :], in1=t_dd[:, :])

            def v_phase(j):
                t_d2 = d2[j]
                if j == 0:  # v carry seed
                    nc.vector.tensor_copy(out=t_d2[0:1, 0:LC], in_=v0_t[:])
                p = psV_pool.tile([128, 2 * LC], f32, tag="psV01")
                psV[j] = p
                nc.tensor.matmul(p[:, :LC], LV_ev, t_d2[:, :LC], start=True, stop=True)
                nc.tensor.matmul(p[:, LC:], LV_x, t_d2[:, :LC], start=True, stop=False)
                nc.tensor.matmul(p[:, LC:], LV_od, t_d2[:, LC:], start=False, stop=True)
                if j + 1 < NPAIR:  # v carry -> row0 of next pair's d2 (over sq's row0)
                    nc.vector.tensor_copy(
                        out=d2[j + 1][0:1, 0:LC], in_=p[0:1, LC : 2 * LC]
                    )
                r01 = r_pool.tile([128, 2 * LC], f16, tag="r01")
                _raw_act(
                    nc.scalar, r01[:, :], p[:, :],
                    mybir.ActivationFunctionType.Rsqrt, eps_t[:, :], 1.0, mybir,
                )
                nc.gpsimd.tensor_mul(
                    out=W[:, 2 * j * LC : (2 * j + 2) * LC],
                    in0=dd[j][:, :], in1=r01[:, :],
                )
                emit_out_dma(2 * j + 2)

            for step in range(NPAIR + 2):
                jj = step - 2
                if 0 <= jj < NPAIR:
                    v_phase(jj)
                if step < NPAIR:
                    d_phase(step)

            # tail: block 64 (64 rows; pretile zero-padded the rest)
            st = slice(64 * LC, 65 * LC)
            nc.vector.tensor_copy(out=W[0:1, st], in_=psD[NPAIR - 1][0:1, LC : 2 * LC])
            pDt = psD_pool.tile([128, 2 * LC], f32, tag="psD01")
            nc.tensor.matmul(pDt[:, :LC], LD_ev, W[:, st], start=True, stop=True)
            ddt = dd_pool.tile([128, 2 * LC], f16, tag="dd01")
            nc.scalar.copy(out=ddt[:, :LC], in_=pDt[:, :LC])
            d2t = d2_pool.tile([128, 2 * LC], f16, tag="d201")
            nc.vector.tensor_mul(out=d2t[:, :LC], in0=ddt[:, :LC], in1=ddt[:, :LC])
            nc.vector.tensor_copy(
                out=d2t[0:1, 0:LC], in_=psV[NPAIR - 1][0:1, LC : 2 * LC]
            )
            pVt = psV_pool.tile([128, 2 * LC], f32, tag="psV01")
            nc.tensor.matmul(pVt[:, :LC], LV_ev, d2t[:, :LC], start=True, stop=True)
            rt = r_pool.tile([128, 2 * LC], f16, tag="r01")
            _raw_act(
                nc.scalar, rt[:, :LC], pVt[:, :LC],
                mybir.ActivationFunctionType.Rsqrt, eps_t[:, :], 1.0, mybir,
            )
            nc.vector.tensor_mul(out=W[:, st], in0=ddt[:, :LC], in1=rt[:, :LC])
            emit_out_dma(NBLK)

    nc.compile()
    _PROGRAM_CACHE["nc"] = nc
    return nc


def kernel(x: np.ndarray, m: np.ndarray, var: np.ndarray) -> np.ndarray:
    from concourse.bass_utils import run_bass_kernel_spmd

    x = np.asarray(x, dtype=_f32)
    m = np.ascontiguousarray(np.asarray(m, dtype=_f32))
    var = np.ascontiguousarray(np.asarray(var, dtype=_f32))
    assert x.shape == (N_ROWS, L_FULL), x.shape

    nc = _build_program()
    mats = _build_mats()

    x_bf = x.astype(_f16)
    negm_bf = (-m).astype(_f16)
    var_bf = var.astype(_f16)

    in_maps = []
    for c in range(N_CORES):
        sl = slice(c * LC, (c + 1) * LC)
        in_maps.append(
            {
                "xp": _pretile(np.ascontiguousarray(x_bf[:, sl]), negm_bf[sl]),
                "var": np.ascontiguousarray(var_bf[sl]),
                "mats": mats,
            }
        )

    res = run_bass_kernel_spmd(nc, in_maps, core_ids=list(range(N_CORES)))
    out = np.concatenate(
        [_unpretile(res.results[c]["op"]) for c in range(N_CORES)], axis=1
    )
    return out.astype(_f32, copy=False)
